# revision 20
# baseline (speedup 1.0000x reference)
"""Trainium2 Bass kernel for chunked decayed outer-product state accumulation.

Math (per batch b, head h):
    out[b,h,p,n] = sum_t exp(sum_{t'>t} A[b,t',h]) * X[b,t,h,p] * B[b,t,h,n]

which is exactly the reference's chunked cumsum/exp/einsum pipeline collapsed
into a single decay-weighted contraction over the full sequence.

Strategy:
  - Host precomputes the decay weights W[b,t,h] = exp(total - cumsum(A))[b,t,h]
    (O(A) work, 4 MiB of 516 MiB of input traffic; HBM bytes are unchanged
    since W replaces A as the kernel input).
  - 8 cores <- 8 batches (data parallel over batch; each core does all 16
    heads, contiguous 64 MiB of X+B per core).
  - Per core: stream X and B in 4 MiB chunks ([128 t x 8 subtiles x 1024]),
    scale X in place by the per-(t,h) decay (split across Vector and Scalar
    engines), then per (t-tile, head) a 128x64x64 fp32 matmul accumulating
    into PSUM over the full sequence. Heads 0-7 accumulate in PSUM bank A,
    heads 8-15 in bank B (one accumulation-group start per bank).
  - Final [64, 1024] result is copied to SBUF, DMA'd out, and the host
    transposes (p,h,n) -> (h,p,n) and stacks batches.
"""

import numpy as np

BATCH, SEQ, H, P, N, L = 8, 8192, 16, 64, 64, 64
HD = H * P  # 1024 floats per t row
T_TILE = 128  # contraction tile (SBUF partitions)

_cache = {}


def _split_plan(n_ttiles, body):
    """Uniform big body chunks (best DMA stream rate) + a resident tail of
    small pieces with dedicated buffers, so the end-of-stream compute lag is
    one small piece instead of one whole chunk."""
    if n_ttiles >= 4 * body:
        tail = n_ttiles % body
        while tail < body + 2:
            tail += body
        assert tail % 2 == 0
        pieces = [2] * (tail // 2)
    else:
        tail, pieces = 0, []
    nbody = (n_ttiles - tail) // body
    return [body] * nbody, pieces


def _build(seq, tiles_per_chunk):
    import concourse.bacc as bacc
    import concourse.mybir as mybir
    import concourse.tile as tile

    f32 = mybir.dt.float32
    n_ttiles = seq // T_TILE
    body_plan, tail_pieces = _split_plan(n_ttiles, tiles_per_chunk)
    n_tail = sum(tail_pieces)

    nc = bacc.Bacc(None, target_bir_lowering=False, enable_partition_id=False)
    Xd = nc.dram_tensor("x", [seq, HD], f32, kind="ExternalInput")
    Bd = nc.dram_tensor("bmat", [seq, HD], f32, kind="ExternalInput")
    Wd = nc.dram_tensor("w", [T_TILE, n_ttiles, H], f32, kind="ExternalInput")
    Od = nc.dram_tensor("out", [P, 2, 8, N], f32, kind="ExternalOutput")

    Xv = Xd.rearrange("(i tp) f -> i tp f", tp=T_TILE)  # [n_ttiles, 128, 1024]
    Bv = Bd.rearrange("(i tp) f -> i tp f", tp=T_TILE)

    def chunk_ap(view, it0, nt):
        # [128, nt, 1024] strided view covering t-tiles it0 .. it0+nt-1
        v = view[it0 : it0 + nt]  # [nt, 128, 1024]
        return v.rearrange("i tp f -> tp i f")

    with tile.TileContext(nc) as tc:
        with (
            tc.tile_pool(name="xp", bufs=2) as xp,
            tc.tile_pool(name="bp", bufs=2) as bp,
            tc.tile_pool(name="singles", bufs=1) as singles,
            tc.tile_pool(name="psum", bufs=1, space="PSUM") as psum_pool,
        ):
            w_sb = singles.tile([T_TILE, n_ttiles, H], f32)
            nc.sync.dma_start(out=w_sb[:], in_=Wd[:])

            ps = [
                psum_pool.tile([P, 8, N], f32, tag=f"ps{i}", name=f"ps{i}")
                for i in range(2)
            ]

            def scale_and_matmul(tile_x, tile_b, s, it):
                for h in range(H):
                    xs = tile_x[:, s, h * P : (h + 1) * P]
                    col = w_sb[:, it, h : h + 1]
                    # 11 heads on DVE, 5 on ACT
                    if (h % 8) < 5 or h == 13:
                        nc.vector.tensor_scalar_mul(xs, xs, col)
                    else:
                        nc.scalar.activation(
                            out=xs,
                            in_=xs,
                            func=mybir.ActivationFunctionType.Copy,
                            scale=col,
                        )
                for h in range(H):
                    bank, slot = divmod(h, 8)
                    nc.tensor.matmul(
                        ps[bank][:, slot, :],
                        tile_x[:, s, h * P : (h + 1) * P],
                        tile_b[:, s, h * N : (h + 1) * N],
                        start=(it == 0 and slot == 0),
                        stop=(it == n_ttiles - 1 and slot == 7),
                    )

            it0 = 0
            for nt in body_plan:
                x_t = xp.tile([T_TILE, tiles_per_chunk, HD], f32, tag="x_t", name="x_t")
                b_t = bp.tile([T_TILE, tiles_per_chunk, HD], f32, tag="b_t", name="b_t")
                nc.sync.dma_start(out=x_t[:, :nt], in_=chunk_ap(Xv, it0, nt))
                nc.sync.dma_start(out=b_t[:, :nt], in_=chunk_ap(Bv, it0, nt))
                for s in range(nt):
                    scale_and_matmul(x_t, b_t, s, it0 + s)
                it0 += nt

            if n_tail:
                # resident tail: dedicated buffers, small DMA pieces issued
                # last in the stream; compute tracks each piece's arrival
                x_tl = singles.tile([T_TILE, n_tail, HD], f32)
                b_tl = singles.tile([T_TILE, n_tail, HD], f32)
                k = 0
                for w in tail_pieces:
                    # x via the SP HWDGE ring, b via the ACT HWDGE ring so the
                    # end-of-stream pieces drain from two independent rings
                    nc.sync.dma_start(
                        out=x_tl[:, k : k + w], in_=chunk_ap(Xv, it0 + k, w)
                    )
                    nc.scalar.dma_start(
                        out=b_tl[:, k : k + w], in_=chunk_ap(Bv, it0 + k, w)
                    )
                    for s in range(k, k + w):
                        scale_and_matmul(x_tl, b_tl, s, it0 + s)
                    k += w
                it0 += n_tail

            out_sb = singles.tile([P, 2, 8, N], f32)
            nc.vector.tensor_copy(out=out_sb[:, 0], in_=ps[0][:])
            nc.sync.dma_start(out=Od[:, 0], in_=out_sb[:, 0])
            nc.vector.tensor_copy(out=out_sb[:, 1], in_=ps[1][:])
            nc.sync.dma_start(out=Od[:, 1], in_=out_sb[:, 1])

    nc.compile()
    return nc


def _get_nc(seq=SEQ, tiles_per_chunk=8):
    key = (seq, tiles_per_chunk)
    if key not in _cache:
        _cache[key] = _build(seq, tiles_per_chunk)
    return _cache[key]


def _decay_weights(A):
    # W[b,t,h] = exp(sum_{t'>t} A[b,t',h]), computed in f64 on host.
    cs = np.cumsum(A.astype(np.float64), axis=1)
    W = np.exp(cs[:, -1:, :] - cs).astype(np.float32)  # (b, s, h)
    b, s, h = W.shape
    # rearrange to (b, 128, n_ttiles, H): W_r[b, tp, i, h] = W[b, i*128+tp, h]
    W_r = np.ascontiguousarray(
        W.reshape(b, s // T_TILE, T_TILE, h).transpose(0, 2, 1, 3)
    )
    return W_r


def run(X, A, B, trace=False, tiles_per_chunk=6, **spmd_kwargs):
    from concourse.bass_utils import run_bass_kernel_spmd

    X = np.asarray(X)
    A = np.asarray(A)
    B = np.asarray(B)
    b, s, h, p = X.shape
    nc = _get_nc(seq=s, tiles_per_chunk=tiles_per_chunk)
    W_r = _decay_weights(A)
    n_ttiles = s // T_TILE

    in_maps = [
        {
            "x": X[i].reshape(s, HD),
            "bmat": B[i].reshape(s, HD),
            "w": W_r[i].reshape(T_TILE, n_ttiles, H),
        }
        for i in range(b)
    ]
    res = run_bass_kernel_spmd(
        nc, in_maps, core_ids=list(range(b)), trace=trace, **spmd_kwargs
    )
    outs = [
        r["out"].reshape(P, H, N).transpose(1, 0, 2) for r in res.results
    ]  # each (H, P, N)
    out = np.stack(outs).astype(np.float32)  # (b, H, P, N)
    return out, res


def kernel(X, A, B):
    out, _ = run(X, A, B, trace=False)
    return out


# revision 23
# speedup vs baseline: 1.0219x; 1.0219x over previous
"""Trainium2 Bass kernel for chunked decayed outer-product state accumulation.

Math (per batch b, head h):
    out[b,h,p,n] = sum_t exp(sum_{t'>t} A[b,t',h]) * X[b,t,h,p] * B[b,t,h,n]

which is exactly the reference's chunked cumsum/exp/einsum pipeline collapsed
into a single decay-weighted contraction over the full sequence.

Strategy:
  - Host precomputes the decay weights W[b,t,h] = exp(total - cumsum(A))[b,t,h]
    (O(A) work, 4 MiB of 516 MiB of input traffic; HBM bytes are unchanged
    since W replaces A as the kernel input).
  - 8 cores <- 8 batches (data parallel over batch; each core does all 16
    heads, contiguous 64 MiB of X+B per core).
  - Per core: stream X and B in 4 MiB chunks ([128 t x 8 subtiles x 1024]),
    scale X in place by the per-(t,h) decay (split across Vector and Scalar
    engines), then per (t-tile, head) a 128x64x64 fp32 matmul accumulating
    into PSUM over the full sequence. Heads 0-7 accumulate in PSUM bank A,
    heads 8-15 in bank B (one accumulation-group start per bank).
  - Final [64, 1024] result is copied to SBUF, DMA'd out, and the host
    transposes (p,h,n) -> (h,p,n) and stacks batches.
"""

import numpy as np

BATCH, SEQ, H, P, N, L = 8, 8192, 16, 64, 64, 64
HD = H * P  # 1024 floats per t row
T_TILE = 128  # contraction tile (SBUF partitions)

_cache = {}


def _split_plan(n_ttiles, body):
    """Uniform big body chunks (best DMA stream rate) + a resident tail of
    small pieces with dedicated buffers, so the end-of-stream compute lag is
    one small piece instead of one whole chunk."""
    if n_ttiles >= 4 * body:
        tail = n_ttiles % body
        while tail < body + 2:
            tail += body
        assert tail % 2 == 0
        pieces = [2] * (tail // 2)
    else:
        tail, pieces = 0, []
    nbody = (n_ttiles - tail) // body
    return [body] * nbody, pieces


def _build(seq, tiles_per_chunk):
    import concourse.bacc as bacc
    import concourse.bass as bass
    import concourse.mybir as mybir
    import concourse.tile as tile

    f32 = mybir.dt.float32
    n_ttiles = seq // T_TILE
    body_plan, tail_pieces = _split_plan(n_ttiles, tiles_per_chunk)
    n_tail = sum(tail_pieces)

    nc = bacc.Bacc(None, target_bir_lowering=False, enable_partition_id=False)
    Xd = nc.dram_tensor("x", [seq, HD], f32, kind="ExternalInput")
    Bd = nc.dram_tensor("bmat", [seq, HD], f32, kind="ExternalInput")
    Wd = nc.dram_tensor("w", [T_TILE, n_ttiles, H], f32, kind="ExternalInput")
    Od = nc.dram_tensor("out", [P, 2, 8, N], f32, kind="ExternalOutput")

    Xv = Xd.rearrange("(i tp) f -> i tp f", tp=T_TILE)  # [n_ttiles, 128, 1024]
    Bv = Bd.rearrange("(i tp) f -> i tp f", tp=T_TILE)

    def chunk_ap(view, it0, nt):
        # [128, nt, 1024] strided view covering t-tiles it0 .. it0+nt-1
        v = view[it0 : it0 + nt]  # [nt, 128, 1024]
        return v.rearrange("i tp f -> tp i f")

    with tile.TileContext(nc) as tc:
        with (
            tc.tile_pool(name="xp", bufs=2) as xp,
            tc.tile_pool(name="bp", bufs=2) as bp,
            tc.tile_pool(name="singles", bufs=1) as singles,
            tc.tile_pool(name="psum", bufs=1, space="PSUM") as psum_pool,
        ):
            w_sb = singles.tile([T_TILE, n_ttiles, H], f32)
            nc.sync.dma_start(out=w_sb[:], in_=Wd[:])

            ps = [
                psum_pool.tile([P, 8, N], f32, tag=f"ps{i}", name=f"ps{i}")
                for i in range(2)
            ]

            def scale_and_matmul(tile_x, tile_b, s, it):
                # one broadcast multiply per subtile: x[:, s, (h,p)] *= w[:, it, h]
                xs3 = tile_x[:, s].rearrange("tp (h p) -> tp h p", h=H)
                wcol = w_sb[:, it]  # [128, H]
                wb = bass.AP(
                    tensor=wcol.tensor,
                    offset=wcol.offset,
                    ap=[wcol.ap[0], wcol.ap[1], [0, P]],
                )
                nc.vector.tensor_mul(xs3, xs3, wb)
                for h in range(H):
                    bank, slot = divmod(h, 8)
                    nc.tensor.matmul(
                        ps[bank][:, slot, :],
                        tile_x[:, s, h * P : (h + 1) * P],
                        tile_b[:, s, h * N : (h + 1) * N],
                        start=(it == 0 and slot == 0),
                        stop=(it == n_ttiles - 1 and slot == 7),
                    )

            it0 = 0
            for nt in body_plan:
                x_t = xp.tile([T_TILE, tiles_per_chunk, HD], f32, tag="x_t", name="x_t")
                b_t = bp.tile([T_TILE, tiles_per_chunk, HD], f32, tag="b_t", name="b_t")
                nc.sync.dma_start(out=x_t[:, :nt], in_=chunk_ap(Xv, it0, nt))
                nc.sync.dma_start(out=b_t[:, :nt], in_=chunk_ap(Bv, it0, nt))
                for s in range(nt):
                    scale_and_matmul(x_t, b_t, s, it0 + s)
                it0 += nt

            if n_tail:
                # resident tail: dedicated buffers, small DMA pieces issued
                # last in the stream; compute tracks each piece's arrival
                x_tl = singles.tile([T_TILE, n_tail, HD], f32)
                b_tl = singles.tile([T_TILE, n_tail, HD], f32)
                k = 0
                for w in tail_pieces:
                    nc.sync.dma_start(
                        out=x_tl[:, k : k + w], in_=chunk_ap(Xv, it0 + k, w)
                    )
                    nc.sync.dma_start(
                        out=b_tl[:, k : k + w], in_=chunk_ap(Bv, it0 + k, w)
                    )
                    for s in range(k, k + w):
                        scale_and_matmul(x_tl, b_tl, s, it0 + s)
                    k += w
                it0 += n_tail

            out_sb = singles.tile([P, 2, 8, N], f32)
            nc.vector.tensor_copy(out=out_sb[:, 0], in_=ps[0][:])
            nc.sync.dma_start(out=Od[:, 0], in_=out_sb[:, 0])
            nc.vector.tensor_copy(out=out_sb[:, 1], in_=ps[1][:])
            nc.sync.dma_start(out=Od[:, 1], in_=out_sb[:, 1])

    nc.compile()
    return nc


def _get_nc(seq=SEQ, tiles_per_chunk=8):
    key = (seq, tiles_per_chunk)
    if key not in _cache:
        _cache[key] = _build(seq, tiles_per_chunk)
    return _cache[key]


def _decay_weights(A):
    # W[b,t,h] = exp(sum_{t'>t} A[b,t',h]), computed in f64 on host.
    cs = np.cumsum(A.astype(np.float64), axis=1)
    W = np.exp(cs[:, -1:, :] - cs).astype(np.float32)  # (b, s, h)
    b, s, h = W.shape
    # rearrange to (b, 128, n_ttiles, H): W_r[b, tp, i, h] = W[b, i*128+tp, h]
    W_r = np.ascontiguousarray(
        W.reshape(b, s // T_TILE, T_TILE, h).transpose(0, 2, 1, 3)
    )
    return W_r


def run(X, A, B, trace=False, tiles_per_chunk=6, **spmd_kwargs):
    from concourse.bass_utils import run_bass_kernel_spmd

    X = np.asarray(X)
    A = np.asarray(A)
    B = np.asarray(B)
    b, s, h, p = X.shape
    nc = _get_nc(seq=s, tiles_per_chunk=tiles_per_chunk)
    W_r = _decay_weights(A)
    n_ttiles = s // T_TILE

    in_maps = [
        {
            "x": X[i].reshape(s, HD),
            "bmat": B[i].reshape(s, HD),
            "w": W_r[i].reshape(T_TILE, n_ttiles, H),
        }
        for i in range(b)
    ]
    res = run_bass_kernel_spmd(
        nc, in_maps, core_ids=list(range(b)), trace=trace, **spmd_kwargs
    )
    outs = [
        r["out"].reshape(P, H, N).transpose(1, 0, 2) for r in res.results
    ]  # each (H, P, N)
    out = np.stack(outs).astype(np.float32)  # (b, H, P, N)
    return out, res


def kernel(X, A, B):
    out, _ = run(X, A, B, trace=False)
    return out


# revision 25
# speedup vs baseline: 1.0531x; 1.0305x over previous
"""Trainium2 Bass kernel for chunked decayed outer-product state accumulation.

Math (per batch b, head h):
    out[b,h,p,n] = sum_t exp(sum_{t'>t} A[b,t',h]) * X[b,t,h,p] * B[b,t,h,n]

which is exactly the reference's chunked cumsum/exp/einsum pipeline collapsed
into a single decay-weighted contraction over the full sequence.

Strategy:
  - Host precomputes the decay weights W[b,t,h] = exp(total - cumsum(A))[b,t,h]
    (O(A) work, 4 MiB of 516 MiB of input traffic; HBM bytes are unchanged
    since W replaces A as the kernel input).
  - 8 cores <- 8 batches (data parallel over batch; each core does all 16
    heads, contiguous 64 MiB of X+B per core).
  - Per core: stream X and B in 4 MiB chunks ([128 t x 8 subtiles x 1024]),
    scale X in place by the per-(t,h) decay (split across Vector and Scalar
    engines), then per (t-tile, head) a 128x64x64 fp32 matmul accumulating
    into PSUM over the full sequence. Heads 0-7 accumulate in PSUM bank A,
    heads 8-15 in bank B (one accumulation-group start per bank).
  - Final [64, 1024] result is copied to SBUF, DMA'd out, and the host
    transposes (p,h,n) -> (h,p,n) and stacks batches.
"""

import numpy as np

BATCH, SEQ, H, P, N, L = 8, 8192, 16, 64, 64, 64
HD = H * P  # 1024 floats per t row
T_TILE = 128  # contraction tile (SBUF partitions)

_cache = {}


def _split_plan(n_ttiles, body):
    """Uniform big body chunks (best DMA stream rate) + a resident tail of
    small pieces with dedicated buffers, so the end-of-stream compute lag is
    one small piece instead of one whole chunk."""
    if n_ttiles >= 4 * body:
        tail = n_ttiles % body
        while tail < 8:
            tail += body
        pieces = [2] * ((tail - 4) // 2) + [2, 1, 1]
        assert sum(pieces) == tail
    else:
        tail, pieces = 0, []
    nbody = (n_ttiles - tail) // body
    return [body] * nbody, pieces


def _build(seq, tiles_per_chunk):
    import concourse.bacc as bacc
    import concourse.bass as bass
    import concourse.mybir as mybir
    import concourse.tile as tile

    f32 = mybir.dt.float32
    n_ttiles = seq // T_TILE
    body_plan, tail_pieces = _split_plan(n_ttiles, tiles_per_chunk)
    n_tail = sum(tail_pieces)

    nc = bacc.Bacc(None, target_bir_lowering=False, enable_partition_id=False)
    Xd = nc.dram_tensor("x", [seq, HD], f32, kind="ExternalInput")
    Bd = nc.dram_tensor("bmat", [seq, HD], f32, kind="ExternalInput")
    Wd = nc.dram_tensor("w", [T_TILE, n_ttiles, H], f32, kind="ExternalInput")
    Od = nc.dram_tensor("out", [P, 2, 8, N], f32, kind="ExternalOutput")

    Xv = Xd.rearrange("(i tp) f -> i tp f", tp=T_TILE)  # [n_ttiles, 128, 1024]
    Bv = Bd.rearrange("(i tp) f -> i tp f", tp=T_TILE)

    def chunk_ap(view, it0, nt):
        # [128, nt, 1024] strided view covering t-tiles it0 .. it0+nt-1
        v = view[it0 : it0 + nt]  # [nt, 128, 1024]
        return v.rearrange("i tp f -> tp i f")

    with tile.TileContext(nc) as tc:
        with (
            tc.tile_pool(name="xp", bufs=2) as xp,
            tc.tile_pool(name="bp", bufs=2) as bp,
            tc.tile_pool(name="singles", bufs=1) as singles,
            tc.tile_pool(name="psum", bufs=1, space="PSUM") as psum_pool,
        ):
            w_sb = singles.tile([T_TILE, n_ttiles, H], f32)
            nc.sync.dma_start(out=w_sb[:], in_=Wd[:])

            ps = [
                psum_pool.tile([P, 8, N], f32, tag=f"ps{i}", name=f"ps{i}")
                for i in range(2)
            ]

            def scale_and_matmul(tile_x, tile_b, s, it):
                # one broadcast multiply per subtile: x[:, s, (h,p)] *= w[:, it, h]
                xs3 = tile_x[:, s].rearrange("tp (h p) -> tp h p", h=H)
                wcol = w_sb[:, it]  # [128, H]
                wb = bass.AP(
                    tensor=wcol.tensor,
                    offset=wcol.offset,
                    ap=[wcol.ap[0], wcol.ap[1], [0, P]],
                )
                nc.vector.tensor_mul(xs3, xs3, wb)
                for h in range(H):
                    bank, slot = divmod(h, 8)
                    nc.tensor.matmul(
                        ps[bank][:, slot, :],
                        tile_x[:, s, h * P : (h + 1) * P],
                        tile_b[:, s, h * N : (h + 1) * N],
                        start=(it == 0 and slot == 0),
                        stop=(it == n_ttiles - 1 and slot == 7),
                    )

            it0 = 0
            for nt in body_plan:
                x_t = xp.tile([T_TILE, tiles_per_chunk, HD], f32, tag="x_t", name="x_t")
                b_t = bp.tile([T_TILE, tiles_per_chunk, HD], f32, tag="b_t", name="b_t")
                nc.sync.dma_start(out=x_t[:, :nt], in_=chunk_ap(Xv, it0, nt))
                nc.sync.dma_start(out=b_t[:, :nt], in_=chunk_ap(Bv, it0, nt))
                for s in range(nt):
                    scale_and_matmul(x_t, b_t, s, it0 + s)
                it0 += nt

            if n_tail:
                # resident tail: dedicated buffers, small DMA pieces issued
                # last in the stream; compute tracks each piece's arrival
                x_tl = singles.tile([T_TILE, n_tail, HD], f32)
                b_tl = singles.tile([T_TILE, n_tail, HD], f32)
                k = 0
                for w in tail_pieces:
                    nc.sync.dma_start(
                        out=x_tl[:, k : k + w], in_=chunk_ap(Xv, it0 + k, w)
                    )
                    nc.sync.dma_start(
                        out=b_tl[:, k : k + w], in_=chunk_ap(Bv, it0 + k, w)
                    )
                    for s in range(k, k + w):
                        scale_and_matmul(x_tl, b_tl, s, it0 + s)
                    k += w
                it0 += n_tail

            out_sb = singles.tile([P, 2, 8, N], f32)
            nc.vector.tensor_copy(out=out_sb[:, 0], in_=ps[0][:])
            nc.sync.dma_start(out=Od[:, 0], in_=out_sb[:, 0])
            nc.vector.tensor_copy(out=out_sb[:, 1], in_=ps[1][:])
            nc.sync.dma_start(out=Od[:, 1], in_=out_sb[:, 1])

    nc.compile()
    return nc


def _get_nc(seq=SEQ, tiles_per_chunk=8):
    key = (seq, tiles_per_chunk)
    if key not in _cache:
        _cache[key] = _build(seq, tiles_per_chunk)
    return _cache[key]


def _decay_weights(A):
    # W[b,t,h] = exp(sum_{t'>t} A[b,t',h]), computed in f64 on host.
    cs = np.cumsum(A.astype(np.float64), axis=1)
    W = np.exp(cs[:, -1:, :] - cs).astype(np.float32)  # (b, s, h)
    b, s, h = W.shape
    # rearrange to (b, 128, n_ttiles, H): W_r[b, tp, i, h] = W[b, i*128+tp, h]
    W_r = np.ascontiguousarray(
        W.reshape(b, s // T_TILE, T_TILE, h).transpose(0, 2, 1, 3)
    )
    return W_r


def run(X, A, B, trace=False, tiles_per_chunk=8, **spmd_kwargs):
    from concourse.bass_utils import run_bass_kernel_spmd

    X = np.asarray(X)
    A = np.asarray(A)
    B = np.asarray(B)
    b, s, h, p = X.shape
    nc = _get_nc(seq=s, tiles_per_chunk=tiles_per_chunk)
    W_r = _decay_weights(A)
    n_ttiles = s // T_TILE

    in_maps = [
        {
            "x": X[i].reshape(s, HD),
            "bmat": B[i].reshape(s, HD),
            "w": W_r[i].reshape(T_TILE, n_ttiles, H),
        }
        for i in range(b)
    ]
    res = run_bass_kernel_spmd(
        nc, in_maps, core_ids=list(range(b)), trace=trace, **spmd_kwargs
    )
    outs = [
        r["out"].reshape(P, H, N).transpose(1, 0, 2) for r in res.results
    ]  # each (H, P, N)
    out = np.stack(outs).astype(np.float32)  # (b, H, P, N)
    return out, res


def kernel(X, A, B):
    out, _ = run(X, A, B, trace=False)
    return out


# revision 27
# speedup vs baseline: 1.1307x; 1.0737x over previous
"""Trainium2 Bass kernel for chunked decayed outer-product state accumulation.

Math (per batch b, head h):
    out[b,h,p,n] = sum_t exp(sum_{t'>t} A[b,t',h]) * X[b,t,h,p] * B[b,t,h,n]

which is exactly the reference's chunked cumsum/exp/einsum pipeline collapsed
into a single decay-weighted contraction over the full sequence.

Strategy:
  - Host precomputes the decay weights W[b,t,h] = exp(total - cumsum(A))[b,t,h]
    (O(A) work, 4 MiB of 516 MiB of input traffic; HBM bytes are unchanged
    since W replaces A as the kernel input).
  - 8 cores <- 8 batches (data parallel over batch; each core does all 16
    heads, contiguous 64 MiB of X+B per core).
  - Per core: stream X and B in 4 MiB chunks ([128 t x 8 subtiles x 1024]),
    scale X in place by the per-(t,h) decay (split across Vector and Scalar
    engines), then per (t-tile, head) a 128x64x64 fp32 matmul accumulating
    into PSUM over the full sequence. Heads 0-7 accumulate in PSUM bank A,
    heads 8-15 in bank B (one accumulation-group start per bank).
  - Final [64, 1024] result is copied to SBUF, DMA'd out, and the host
    transposes (p,h,n) -> (h,p,n) and stacks batches.
"""

import numpy as np

BATCH, SEQ, H, P, N, L = 8, 8192, 16, 64, 64, 64
HD = H * P  # 1024 floats per t row
T_TILE = 128  # contraction tile (SBUF partitions)

_cache = {}


def _split_plan(n_ttiles, body):
    """Uniform big body chunks (best DMA stream rate) + a resident tail of
    small pieces with dedicated buffers, so the end-of-stream compute lag is
    one small piece instead of one whole chunk."""
    if n_ttiles >= 4 * body:
        tail = n_ttiles % body
        while tail < 8:
            tail += body
        pieces = [2] * ((tail - 4) // 2) + [2, 1, 1]
        assert sum(pieces) == tail
    else:
        tail, pieces = 0, []
    nbody = (n_ttiles - tail) // body
    return [body] * nbody, pieces


def _build(seq, tiles_per_chunk):
    import concourse.bacc as bacc
    import concourse.bass as bass
    import concourse.mybir as mybir
    import concourse.tile as tile

    f32 = mybir.dt.float32
    n_ttiles = seq // T_TILE
    body_plan, tail_pieces = _split_plan(n_ttiles, tiles_per_chunk)
    n_tail = sum(tail_pieces)

    nc = bacc.Bacc(None, target_bir_lowering=False, enable_partition_id=False)
    Xd = nc.dram_tensor("x", [seq, HD], f32, kind="ExternalInput")
    Bd = nc.dram_tensor("bmat", [seq, HD], f32, kind="ExternalInput")
    Wd = nc.dram_tensor("w", [T_TILE, n_ttiles, H], f32, kind="ExternalInput")
    Od = nc.dram_tensor("out", [P, 2, 8, N], f32, kind="ExternalOutput")

    Xv = Xd.rearrange("(i tp) f -> i tp f", tp=T_TILE)  # [n_ttiles, 128, 1024]
    Bv = Bd.rearrange("(i tp) f -> i tp f", tp=T_TILE)

    def chunk_ap(view, it0, nt):
        # [128, nt, 1024] strided view covering t-tiles it0 .. it0+nt-1
        v = view[it0 : it0 + nt]  # [nt, 128, 1024]
        return v.rearrange("i tp f -> tp i f")

    with tile.TileContext(nc) as tc:
        with (
            tc.tile_pool(name="xp", bufs=2) as xp,
            tc.tile_pool(name="bp", bufs=2) as bp,
            tc.tile_pool(name="singles", bufs=1) as singles,
            tc.tile_pool(name="psum", bufs=1, space="PSUM") as psum_pool,
        ):
            w_sb = singles.tile([T_TILE, n_ttiles, H], f32)
            nc.sync.dma_start(out=w_sb[:], in_=Wd[:])

            ps = [
                psum_pool.tile([P, 8, N], f32, tag=f"ps{i}", name=f"ps{i}")
                for i in range(2)
            ]

            def scale_and_matmul(tile_x, tile_b, s, it):
                for h in range(H):
                    xs = tile_x[:, s, h * P : (h + 1) * P]
                    col = w_sb[:, it, h : h + 1]
                    # 11 heads on DVE, 5 on ACT
                    if (h % 8) < 5 or h == 13:
                        nc.vector.tensor_scalar_mul(xs, xs, col)
                    else:
                        nc.scalar.activation(
                            out=xs,
                            in_=xs,
                            func=mybir.ActivationFunctionType.Copy,
                            scale=col,
                        )
                for h in range(H):
                    bank, slot = divmod(h, 8)
                    nc.tensor.matmul(
                        ps[bank][:, slot, :],
                        tile_x[:, s, h * P : (h + 1) * P],
                        tile_b[:, s, h * N : (h + 1) * N],
                        start=(it == 0 and slot == 0),
                        stop=(it == n_ttiles - 1 and slot == 7),
                    )

            it0 = 0
            for nt in body_plan:
                x_t = xp.tile([T_TILE, tiles_per_chunk, HD], f32, tag="x_t", name="x_t")
                b_t = bp.tile([T_TILE, tiles_per_chunk, HD], f32, tag="b_t", name="b_t")
                nc.sync.dma_start(out=x_t[:, :nt], in_=chunk_ap(Xv, it0, nt))
                nc.sync.dma_start(out=b_t[:, :nt], in_=chunk_ap(Bv, it0, nt))
                for s in range(nt):
                    scale_and_matmul(x_t, b_t, s, it0 + s)
                it0 += nt

            if n_tail:
                # resident tail: dedicated buffers, small DMA pieces issued
                # last in the stream; compute tracks each piece's arrival
                x_tl = singles.tile([T_TILE, n_tail, HD], f32)
                b_tl = singles.tile([T_TILE, n_tail, HD], f32)
                k = 0
                for w in tail_pieces:
                    nc.sync.dma_start(
                        out=x_tl[:, k : k + w], in_=chunk_ap(Xv, it0 + k, w)
                    )
                    nc.sync.dma_start(
                        out=b_tl[:, k : k + w], in_=chunk_ap(Bv, it0 + k, w)
                    )
                    for s in range(k, k + w):
                        scale_and_matmul(x_tl, b_tl, s, it0 + s)
                    k += w
                it0 += n_tail

            out_sb = singles.tile([P, 2, 8, N], f32)
            nc.vector.tensor_copy(out=out_sb[:, 0], in_=ps[0][:])
            nc.sync.dma_start(out=Od[:, 0], in_=out_sb[:, 0])
            nc.vector.tensor_copy(out=out_sb[:, 1], in_=ps[1][:])
            nc.sync.dma_start(out=Od[:, 1], in_=out_sb[:, 1])

    nc.compile()
    return nc


def _get_nc(seq=SEQ, tiles_per_chunk=8):
    key = (seq, tiles_per_chunk)
    if key not in _cache:
        _cache[key] = _build(seq, tiles_per_chunk)
    return _cache[key]


def _decay_weights(A):
    # W[b,t,h] = exp(sum_{t'>t} A[b,t',h]), computed in f64 on host.
    cs = np.cumsum(A.astype(np.float64), axis=1)
    W = np.exp(cs[:, -1:, :] - cs).astype(np.float32)  # (b, s, h)
    b, s, h = W.shape
    # rearrange to (b, 128, n_ttiles, H): W_r[b, tp, i, h] = W[b, i*128+tp, h]
    W_r = np.ascontiguousarray(
        W.reshape(b, s // T_TILE, T_TILE, h).transpose(0, 2, 1, 3)
    )
    return W_r


def run(X, A, B, trace=False, tiles_per_chunk=6, **spmd_kwargs):
    from concourse.bass_utils import run_bass_kernel_spmd

    X = np.asarray(X)
    A = np.asarray(A)
    B = np.asarray(B)
    b, s, h, p = X.shape
    nc = _get_nc(seq=s, tiles_per_chunk=tiles_per_chunk)
    W_r = _decay_weights(A)
    n_ttiles = s // T_TILE

    in_maps = [
        {
            "x": X[i].reshape(s, HD),
            "bmat": B[i].reshape(s, HD),
            "w": W_r[i].reshape(T_TILE, n_ttiles, H),
        }
        for i in range(b)
    ]
    res = run_bass_kernel_spmd(
        nc, in_maps, core_ids=list(range(b)), trace=trace, **spmd_kwargs
    )
    outs = [
        r["out"].reshape(P, H, N).transpose(1, 0, 2) for r in res.results
    ]  # each (H, P, N)
    out = np.stack(outs).astype(np.float32)  # (b, H, P, N)
    return out, res


def kernel(X, A, B):
    out, _ = run(X, A, B, trace=False)
    return out
